# revision 6
# baseline (speedup 1.0000x reference)
"""MoD wrapper (router -> top-k -> gather -> GELU MLP -> weighted scatter-add)
on 8 Trainium2 NeuronCores.

Sharding: data-parallel over batch (4 sequences) x 2-way split of each
sequence's K=2048 selected tokens -> 8 cores, 1024 tokens each. Every core
holds the full FFN weights (bf16) and computes
    y[t, :] = gate[t] * gelu_tanh(x[t, :] @ w1 + b1) @ w2
for its 1024 tokens. Inputs are pre-scaled by powers of two (SX/SW1/SW2) on
the host so everything sits in e4m3's normal range (max +-240 on TRN); the
dequant 1/(SX*SW1) folds into the gelu activation's input scale and 1/SW2
folds into the per-token gate. Routing (scores / top-k / sigmoid) runs
through the same jax ops the reference uses, so token selection matches the
reference bit-for-bit; gather and the final scatter-add into the residual
stream are host-side numpy (b2 is folded into the scatter).
"""

import sys
import types

import numpy as np
import ml_dtypes

# bass_utils' trace path does `from antenv.axon_hooks import ...`; some
# images ship an antenv without that module (boot degrades silently but the
# import in bass_utils would crash). Register a no-op stand-in so trace=True
# degrades to "no profile" instead of raising.
try:
    import antenv.axon_hooks  # noqa: F401
except Exception:
    import antenv

    _hooks = types.ModuleType("antenv.axon_hooks")
    _hooks._hook = None
    _hooks.set_axon_ntff_profile_hook = \
        lambda h: setattr(_hooks, "_hook", h)
    _hooks.get_axon_ntff_profile_hook = \
        lambda: getattr(_hooks, "_hook", None)
    sys.modules["antenv.axon_hooks"] = _hooks
    antenv.axon_hooks = _hooks
    try:
        # Same registration trn_agent_boot.boot() would have done had the
        # module existed at interpreter start.
        from trn_agent_boot.trn_boot import _ntff_profile_via_ctypes

        _hook = _ntff_profile_via_ctypes("/opt/axon/libaxon_pjrt.so")
        if _hook is not None:
            _hooks.set_axon_ntff_profile_hook(_hook)
    except Exception:
        pass

import concourse.bacc as bacc
import concourse.bass as bass
import concourse.mybir as mybir
import concourse.tile as tile
from concourse.bass import ts
from concourse.bass_utils import run_bass_kernel_spmd
from concourse.kernels.tile_matmul import (
    ShapeInfo,
    composable_matmul_tile_kernel,
    dma_from_dram_kxm,
    dma_from_dram_kxn,
    dma_to_dram_mxn,
)

B, S, D, DFF = 4, 4096, 2048, 8192
K = 2048          # selected tokens per sequence
N_CORES = 8
TPC = (B * K) // N_CORES  # tokens per core = 1024

BF16 = mybir.dt.bfloat16
FP8 = mybir.dt.float8e4
F32 = mybir.dt.float32
P = 128

# Power-of-two quantization scales keeping |values| inside e4m3's normal
# range (TRN max normal +-240, min normal 2^-6).
SX = 32.0     # hidden_states (std 1, absmax ~5.7 -> ~182)
SW1 = 1024.0  # w1 (std 1/sqrt(2048), absmax ~0.13 -> ~135)
SW2 = 2048.0  # w2 (std 1/sqrt(8192), absmax ~0.065 -> ~134)


def _build_nc():
    nc = bacc.Bacc("TRN2", target_bir_lowering=False, debug=False,
                   num_devices=N_CORES)

    xT_ap = nc.dram_tensor("xT", [D, TPC], FP8, kind="ExternalInput").ap()
    w1_ap = nc.dram_tensor("w1", [D, DFF], FP8, kind="ExternalInput").ap()
    w2_ap = nc.dram_tensor("w2", [DFF, D], FP8, kind="ExternalInput").ap()
    gate_ap = nc.dram_tensor("gate", [P, TPC // P], F32, kind="ExternalInput").ap()
    b1_ap = nc.dram_tensor("b1v", [P, DFF // P], F32, kind="ExternalInput").ap()
    y_ap = nc.dram_tensor("y", [TPC, D], F32, kind="ExternalOutput").ap()

    with tile.TileContext(nc) as tc:
        with (
            tc.tile_pool(name="const", bufs=1) as const_pool,
            tc.tile_pool(name="hT", bufs=1) as hT_pool,
            tc.tile_pool(name="kxm1", bufs=5) as kxm1_pool,
            tc.tile_pool(name="kxn1", bufs=5) as kxn1_pool,
            tc.tile_pool(name="kxn2", bufs=3) as kxn2_pool,
        ):
            gate_sb = const_pool.tile([P, TPC // P], F32)
            b1_sb = const_pool.tile([P, DFF // P], F32)
            nc.gpsimd.dma_start(gate_sb[:], gate_ap[:])
            nc.gpsimd.dma_start(b1_sb[:], b1_ap[:])

            # Intermediate hT[f, t] = gelu(w1.T @ x.T + b1), kept in SBUF
            # as the kxm operand of the second matmul. [128, 64, 1024] fp8.
            hT_cache = hT_pool.tile([P, DFF // P, TPC], FP8)

            # ---- phase 1: hT = gelu(w1.T @ xT + b1) ----
            kxm1_producer, kxm1_shape = dma_from_dram_kxm(kxm1_pool, w1_ap)
            kxn1_producer, kxn1_shape = dma_from_dram_kxn(kxn1_pool, xT_ap)

            def hT_slice_producer(nc_, md):
                return hT_cache[:, ts(md.m_tile_idx, md.m_subtiles), md.n_slice]

            def gelu_reducer(nc_, psum, sbuf, md):
                f_outer = md.m_tile_idx * md.m_subtiles + md.m_subtile_idx
                nc_.scalar.activation(
                    sbuf,
                    psum,
                    mybir.ActivationFunctionType.Gelu_apprx_tanh,
                    bias=b1_sb[:, f_outer:f_outer + 1],
                    scale=1.0 / (SX * SW1),
                )

            composable_matmul_tile_kernel(
                tc,
                kxm_shape=kxm1_shape,
                kxn_shape=kxn1_shape,
                output_type=None,
                kxm_producer=kxm1_producer,
                kxn_producer=kxn1_producer,
                mxn_consumer=lambda nc_, t, md: None,
                mxn_subtile_reducer=gelu_reducer,
                mxn_subtile_producer=hT_slice_producer,
                cache_tiles=True,
            )

            # ---- phase 2: y = (hT.T @ w2) * gate ----
            kxm2_shape = ShapeInfo(pdims=((P, DFF // P),), fdims=(TPC,))

            def hT_kxm_producer(nc_, md):
                return hT_cache[:, ts(md.k_tile_idx, md.k_subtiles),
                                ts(md.m_tile_idx, md.m_tile)]

            kxn2_producer, kxn2_shape = dma_from_dram_kxn(kxn2_pool, w2_ap)

            def gate_reducer(nc_, psum, sbuf, md):
                t_outer = md.m_tile_idx * md.m_subtiles + md.m_subtile_idx
                nc_.vector.tensor_scalar_mul(
                    sbuf, psum, gate_sb[:, t_outer:t_outer + 1])

            composable_matmul_tile_kernel(
                tc,
                kxm_shape=kxm2_shape,
                kxn_shape=kxn2_shape,
                output_type=F32,
                kxm_producer=hT_kxm_producer,
                kxn_producer=kxn2_producer,
                mxn_consumer=dma_to_dram_mxn(y_ap),
                mxn_subtile_reducer=gate_reducer,
                cache_tiles=False,
                psum_n_bufs=2,
            )

    nc.compile()
    return nc


_NC = None


def _routing(hidden_states, router_weight, router_bias):
    """Same ops/backend as the reference => bit-identical selection."""
    import jax
    import jax.numpy as jnp
    scores = jnp.einsum('bsd,d->bs', hidden_states, router_weight) \
        + router_bias[0]
    top_scores, indices = jax.lax.top_k(scores, K)
    weights = jax.nn.sigmoid(top_scores)
    return np.asarray(indices), np.asarray(weights)


def _run(hidden_states, router_weight, router_bias, w1, b1, w2, b2,
         trace=False):
    global _NC
    hidden_states = np.asarray(hidden_states, dtype=np.float32)
    router_weight = np.asarray(router_weight, dtype=np.float32)
    router_bias = np.asarray(router_bias, dtype=np.float32)
    w1 = np.asarray(w1, dtype=np.float32)
    b1 = np.asarray(b1, dtype=np.float32)
    w2 = np.asarray(w2, dtype=np.float32)
    b2 = np.asarray(b2, dtype=np.float32)

    indices, weights = _routing(hidden_states, router_weight, router_bias)

    if _NC is None:
        _NC = _build_nc()

    f8 = ml_dtypes.float8_e4m3

    def q8(a, s):
        return np.clip(a * s, -240.0, 240.0).astype(f8)

    w1_f8 = q8(w1, SW1)
    w2_f8 = q8(w2, SW2)
    b1v = np.ascontiguousarray(b1.reshape(DFF // P, P).T)

    in_maps = []
    core_idx = []  # (b, idx_slice) per core
    for c in range(N_CORES):
        b, h = divmod(c, 2)
        idx_c = indices[b, h * TPC:(h + 1) * TPC]
        gate_c = weights[b, h * TPC:(h + 1) * TPC]
        xT = q8(hidden_states[b, idx_c].T, SX)
        # 1/SW2 dequant for the second matmul folds into the gate
        gate_dev = gate_c * (1.0 / SW2)
        in_maps.append({
            "xT": xT,
            "w1": w1_f8,
            "w2": w2_f8,
            "gate": np.ascontiguousarray(gate_dev.reshape(TPC // P, P).T),
            "b1v": b1v,
        })
        core_idx.append((b, idx_c, gate_c))

    res = run_bass_kernel_spmd(_NC, in_maps, core_ids=list(range(N_CORES)),
                               trace=trace)

    out = hidden_states.copy().reshape(B * S, D)
    b2_nonzero = bool(np.any(b2))
    for c in range(N_CORES):
        b, idx_c, gate_c = core_idx[c]
        y = res.results[c]["y"]
        if b2_nonzero:
            y = y + gate_c[:, None] * b2[None, :]
        out[b * S + idx_c] += y
    return out.reshape(B, S, D), res


def kernel(**inputs):
    return _run(**inputs)[0]



# revision 8
# speedup vs baseline: 1.0429x; 1.0429x over previous
"""MoD wrapper (router -> top-k -> gather -> GELU MLP -> weighted scatter-add)
on 8 Trainium2 NeuronCores.

Sharding: data-parallel over batch (4 sequences) x 2-way split of each
sequence's K=2048 selected tokens -> 8 cores, 1024 tokens each. Every core
holds the full FFN weights (fp8 e4m3, DoubleRow matmul = 2x contraction
per pass) and computes
    yT = w2.T @ gelu_tanh(w1.T @ x.T + b1)        # [D, TPC]
for its 1024 tokens; the per-token gate, the 1/SW2 dequant, b2 and the
scatter-add into the residual stream are applied on the host (gate is a
per-token scalar so it commutes with the second matmul).

Custom loop structure (vs composable_matmul_tile_kernel): each DoubleRow
LDWEIGHTS (256 columns, no FWL -> ~213ns) is amortized over the two N=512
matmuls that share its stationary tile, instead of the 1:1 LDW:MM ratio the
generic kernel emits. Both phases keep their moving operand fully resident
in SBUF (xT: 2 MiB, hT: 8 MiB) and stream only the weights.

Inputs are pre-scaled by powers of two (SX/SW1/SW2) on the host so
everything sits in e4m3's normal range (TRN max normal +-240); the
1/(SX*SW1) dequant folds into the gelu activation's input scale. Routing
(scores / top-k / sigmoid) runs through the same jax ops the reference
uses, so token selection matches the reference bit-for-bit.
"""

import sys
import types

import numpy as np
import ml_dtypes

# bass_utils' trace path does `from antenv.axon_hooks import ...`; some
# images ship an antenv without that module (boot degrades silently but the
# import in bass_utils would crash). Register a no-op stand-in so trace=True
# degrades to "no profile" instead of raising.
try:
    import antenv.axon_hooks  # noqa: F401
except Exception:
    import antenv

    _hooks = types.ModuleType("antenv.axon_hooks")
    _hooks._hook = None
    _hooks.set_axon_ntff_profile_hook = \
        lambda h: setattr(_hooks, "_hook", h)
    _hooks.get_axon_ntff_profile_hook = \
        lambda: getattr(_hooks, "_hook", None)
    sys.modules["antenv.axon_hooks"] = _hooks
    antenv.axon_hooks = _hooks
    try:
        # Same registration trn_agent_boot.boot() would have done had the
        # module existed at interpreter start.
        from trn_agent_boot.trn_boot import _ntff_profile_via_ctypes

        _hook = _ntff_profile_via_ctypes("/opt/axon/libaxon_pjrt.so")
        if _hook is not None:
            _hooks.set_axon_ntff_profile_hook(_hook)
    except Exception:
        pass

import concourse.bacc as bacc
import concourse.mybir as mybir
import concourse.tile as tile
from concourse.bass_utils import run_bass_kernel_spmd

B, S, D, DFF = 4, 4096, 2048, 8192
K = 2048          # selected tokens per sequence
N_CORES = 8
TPC = (B * K) // N_CORES  # tokens per core = 1024

BF16 = mybir.dt.bfloat16
FP8 = mybir.dt.float8e4
F32 = mybir.dt.float32
P = 128
DR = mybir.MatmulPerfMode.DoubleRow

# Power-of-two quantization scales keeping |values| inside e4m3's normal
# range (TRN max normal +-240, min normal 2^-6).
SX = 32.0     # hidden_states (std 1, absmax ~5.4 -> ~174)
SW1 = 1024.0  # w1 (std 1/sqrt(2048), absmax ~0.12 -> ~123)
SW2 = 2048.0  # w2 (std 1/sqrt(8192), absmax ~0.06 -> ~123)

KS1 = D // P          # 16 contraction subtiles, phase 1
KP1 = KS1 // 2        # 8 DoubleRow pairs
KS2 = DFF // P        # 64 contraction subtiles, phase 2
KP2 = KS2 // 2        # 32 DoubleRow pairs
NF_CH = 16            # w1 streamed in 16 chunks of 512 f-columns
F_PER_CH = 4          # 4 f-groups of 128 per chunk
ND_CH = 4             # w2 streamed in 4 chunks of 512 d-columns
D_PER_CH = 4          # 4 d-groups of 128 per chunk
W2_TILES = 4          # w2 chunk split into 4 tiles of 16 k-subtiles
HALF = TPC // 2       # 512-token halves (one psum bank each)


def _build_nc():
    nc = bacc.Bacc("TRN2", target_bir_lowering=False, debug=False,
                   num_devices=N_CORES)

    xT_ap = nc.dram_tensor("xT", [D, TPC], FP8, kind="ExternalInput").ap()
    w1_ap = nc.dram_tensor("w1", [D, DFF], FP8, kind="ExternalInput").ap()
    w2_ap = nc.dram_tensor("w2", [DFF, D], FP8, kind="ExternalInput").ap()
    b1_ap = nc.dram_tensor("b1v", [P, DFF // P], F32, kind="ExternalInput").ap()
    yT_ap = nc.dram_tensor("yT", [D, TPC], BF16, kind="ExternalOutput").ap()

    w1_r = w1_ap.rearrange("(ko p) f -> p ko f", p=P)   # [128, 16, 8192]
    w2_r = w2_ap.rearrange("(ko p) d -> p ko d", p=P)   # [128, 64, 2048]
    xT_r = xT_ap.rearrange("(ko p) t -> p ko t", p=P)   # [128, 16, 1024]

    with tile.TileContext(nc) as tc:
        with (
            tc.tile_pool(name="const", bufs=1) as const_pool,
            tc.tile_pool(name="xT", bufs=1) as xT_pool,
            tc.tile_pool(name="hT", bufs=1) as hT_pool,
            tc.tile_pool(name="w1", bufs=3) as w1_pool,
            tc.tile_pool(name="w2", bufs=8) as w2_pool,
            tc.tile_pool(name="yout", bufs=3) as y_pool,
            tc.tile_pool(name="ps", bufs=3, space="PSUM") as ps_pool,
        ):
            b1_sb = const_pool.tile([P, DFF // P], F32)
            nc.gpsimd.dma_start(b1_sb[:], b1_ap[:])

            xT_sb = xT_pool.tile([P, KS1, TPC], FP8)
            nc.gpsimd.dma_start(xT_sb[:], xT_r[:])

            # Intermediate hT[f, t] = gelu(w1.T @ xT + b1), resident in SBUF
            # as the moving operand of phase 2. [128, 64, 1024] fp8.
            hT = hT_pool.tile([P, KS2, TPC], FP8)

            # ---- phase 1: hT = gelu(w1.T @ xT + b1) ----
            for fch in range(NF_CH):
                w1_t = w1_pool.tile([P, KS1, 512], FP8, name="w1t")
                nc.sync.dma_start(w1_t[:], w1_r[:, :, fch * 512:(fch + 1) * 512])
                for fg in range(F_PER_CH):
                    f_idx = fch * F_PER_CH + fg
                    pa = ps_pool.tile([P, HALF], F32, name="pa")
                    pb = ps_pool.tile([P, HALF], F32, name="pb")
                    for kp in range(KP1):
                        lhsT = w1_t[:, 2 * kp:2 * kp + 2,
                                    fg * P:(fg + 1) * P]
                        nc.tensor.matmul(
                            pa, lhsT, xT_sb[:, 2 * kp:2 * kp + 2, 0:HALF],
                            start=(kp == 0), stop=(kp == KP1 - 1),
                            perf_mode=DR)
                        nc.tensor.matmul(
                            pb, lhsT, xT_sb[:, 2 * kp:2 * kp + 2, HALF:TPC],
                            start=(kp == 0), stop=(kp == KP1 - 1),
                            perf_mode=DR)
                    bias = b1_sb[:, f_idx:f_idx + 1]
                    nc.scalar.activation(
                        hT[:, f_idx, 0:HALF], pa,
                        mybir.ActivationFunctionType.Gelu_apprx_tanh,
                        bias=bias, scale=1.0 / (SX * SW1))
                    nc.scalar.activation(
                        hT[:, f_idx, HALF:TPC], pb,
                        mybir.ActivationFunctionType.Gelu_apprx_tanh,
                        bias=bias, scale=1.0 / (SX * SW1))

            # ---- phase 2: yT = w2.T @ hT  (gate/dequant on host) ----
            for dch in range(ND_CH):
                w2_ts = []
                for j in range(W2_TILES):
                    w2_t = w2_pool.tile([P, KS2 // W2_TILES, 512], FP8,
                                        name="w2t")
                    nc.gpsimd.dma_start(
                        w2_t[:],
                        w2_r[:, j * (KS2 // W2_TILES):(j + 1) * (KS2 // W2_TILES),
                             dch * 512:(dch + 1) * 512])
                    w2_ts.append(w2_t)
                for dg in range(D_PER_CH):
                    d_idx = dch * D_PER_CH + dg
                    pa = ps_pool.tile([P, HALF], F32, name="pa")
                    pb = ps_pool.tile([P, HALF], F32, name="pb")
                    for kp in range(KP2):
                        j, r = divmod(kp, KP2 // W2_TILES)
                        lhsT = w2_ts[j][:, 2 * r:2 * r + 2,
                                        dg * P:(dg + 1) * P]
                        nc.tensor.matmul(
                            pa, lhsT, hT[:, 2 * kp:2 * kp + 2, 0:HALF],
                            start=(kp == 0), stop=(kp == KP2 - 1),
                            perf_mode=DR)
                        nc.tensor.matmul(
                            pb, lhsT, hT[:, 2 * kp:2 * kp + 2, HALF:TPC],
                            start=(kp == 0), stop=(kp == KP2 - 1),
                            perf_mode=DR)
                    y_sb = y_pool.tile([P, TPC], BF16, name="ysb")
                    nc.scalar.copy(y_sb[:, 0:HALF], pa)
                    nc.scalar.copy(y_sb[:, HALF:TPC], pb)
                    nc.sync.dma_start(
                        yT_ap[d_idx * P:(d_idx + 1) * P, :], y_sb[:])

    nc.compile()
    return nc


_NC = None


def _routing(hidden_states, router_weight, router_bias):
    """Same ops/backend as the reference => bit-identical selection."""
    import jax
    import jax.numpy as jnp
    scores = jnp.einsum('bsd,d->bs', hidden_states, router_weight) \
        + router_bias[0]
    top_scores, indices = jax.lax.top_k(scores, K)
    weights = jax.nn.sigmoid(top_scores)
    return np.asarray(indices), np.asarray(weights)


def _run(hidden_states, router_weight, router_bias, w1, b1, w2, b2,
         trace=False):
    global _NC
    hidden_states = np.asarray(hidden_states, dtype=np.float32)
    router_weight = np.asarray(router_weight, dtype=np.float32)
    router_bias = np.asarray(router_bias, dtype=np.float32)
    w1 = np.asarray(w1, dtype=np.float32)
    b1 = np.asarray(b1, dtype=np.float32)
    w2 = np.asarray(w2, dtype=np.float32)
    b2 = np.asarray(b2, dtype=np.float32)

    indices, weights = _routing(hidden_states, router_weight, router_bias)

    if _NC is None:
        _NC = _build_nc()

    f8 = ml_dtypes.float8_e4m3

    def q8(a, s):
        return np.clip(a * s, -240.0, 240.0).astype(f8)

    w1_f8 = q8(w1, SW1)
    w2_f8 = q8(w2, SW2)
    b1v = np.ascontiguousarray(b1.reshape(DFF // P, P).T)

    in_maps = []
    core_idx = []  # (b, idx_slice, gate) per core
    for c in range(N_CORES):
        b, h = divmod(c, 2)
        idx_c = indices[b, h * TPC:(h + 1) * TPC]
        gate_c = weights[b, h * TPC:(h + 1) * TPC]
        xT = q8(hidden_states[b, idx_c].T, SX)
        in_maps.append({
            "xT": xT,
            "w1": w1_f8,
            "w2": w2_f8,
            "b1v": b1v,
        })
        core_idx.append((b, idx_c, gate_c))

    res = run_bass_kernel_spmd(_NC, in_maps, core_ids=list(range(N_CORES)),
                               trace=trace)

    out = hidden_states.copy().reshape(B * S, D)
    b2_nonzero = bool(np.any(b2))
    for c in range(N_CORES):
        b, idx_c, gate_c = core_idx[c]
        yT = res.results[c]["yT"]           # [D, TPC] bf16, = (h @ w2) * SW2
        y = yT.T.astype(np.float32) * (gate_c[:, None] * (1.0 / SW2))
        if b2_nonzero:
            y = y + gate_c[:, None] * b2[None, :]
        out[b * S + idx_c] += y
    return out.reshape(B, S, D), res


def kernel(**inputs):
    return _run(**inputs)[0]
